# revision 12
# baseline (speedup 1.0000x reference)
"""Trainium2 Bass kernel for nn_CrossChannelAttention.

Reference computation (per batch b, pixel p, with C=128 channels, NUMS=16
groups of HEADS=8 channels, OUT=256):
    fm[g,p]  = relu(sum_h W1[g,h] * x[8g+h, p] + b1[g])          # [16, P]
    feat[(g,d), p] = fm[g,p] * x[d,p]                            # [2048, P]
    out[o,p] = sum_c W2[o,c] * feat[c,p] + b2[o]                 # [256, P]

Data-parallel over batch B=8 across the 8 NeuronCores (one image per core,
params replicated).  Per core the PE work (256 accumulating K=128 N=512 bf16
matmuls ~55us warm) is the floor; everything else is scheduled around keeping
the PE saturated at its warm 2.4 GHz p-state:
  - dummy warmup matmuls on a memset scratch tile ramp the PE p-state
    (3us of continuous PE busy -> 2.4 GHz) while the x/W2 DMAs run.
  - fm rows are replicated to 128 partitions by a mix of
    gpsimd.partition_broadcast (groups 11-15, + g0 of the first subchunk)
    and fused DRAM->SBUF broadcast DMAs (one DMA per multi-group chunk via a
    flattened stride-0 view of the fm DRAM buffer).  Fusing amortizes the
    ~600ns per-DMA sequencer issue cost; the gpsimd share keeps total DMA
    traffic (~360 GB/s global across 16 engines) below the PE's pace.
  - feat = x * fm_rep on the vector engine in multi-group ops (stride-0
    broadcast view of x against a [C, ng, w] rep chunk) in bf16 2x mode.
  - the pixel dim is processed as subchunks [512,512,1024,1024,1024]; the
    small leading subchunks shorten the x->fm->broadcast->feat critical
    chain so the first main matmul lands early.
  - per-queue issue order is arranged so DMA transfers hit the (globally
    shared) DMA engines in deadline order; fm writes gate each subchunk's
    broadcast chain and are issued on the otherwise-idle sync queue.
  - outputs are written bf16 (host casts back to fp32), halving store DMA.
Accuracy: bf16 matmuls with fp32 PSUM accumulation; rel err ~4e-3.
"""

import numpy as np
import ml_dtypes

import concourse.bacc as bacc
import concourse.tile as tile
from concourse import mybir
from concourse.bass_utils import run_bass_kernel_spmd

F32 = mybir.dt.float32
BF16 = mybir.dt.bfloat16

B, C, H, W = 8, 128, 64, 64
NUMS, HEADS, OUT = 16, 8, 256
P = H * W          # 4096 pixels per image
PB = 512           # pixel block (one PSUM bank of fp32)
N_CORES = 8

# pixel subchunks (start, width); two small leading chunks for fast fill
SUBS = [(0, 512), (512, 512), (1024, 1024), (2048, 1024), (3072, 1024)]
NSUB = len(SUBS)

# broadcast-DMA group chunks per sub (groups 0..10 unless listed in GP)
BC = {
    0: [(1, 3), (3, 5), (5, 7), (7, 9), (9, 11)],
}
BC_STEADY = [(0, 3), (3, 7), (7, 11)]
# groups replicated by gpsimd.partition_broadcast (from SBUF fm rows)
GP = {0: (0, 11, 12, 13, 14, 15)}
GP_STEADY = (11, 12, 13, 14, 15)
# per-sub feat emission order: ('g', g) gpsimd single, ('d', ci) dma chunk
EMIT = {
    0: [('g', 0), ('d', 0), ('d', 1), ('g', 11), ('d', 2), ('g', 12),
        ('d', 3), ('g', 13), ('d', 4), ('g', 14), ('g', 15)],
}
EMIT_STEADY = [('d', 0), ('d', 1), ('d', 2),
               ('g', 11), ('g', 12), ('g', 13), ('g', 14), ('g', 15)]

NWARM_PRE = 8      # PE warmup matmuls before fm s0 (p-state ramp)
NWARM_MID = 4      # PE filler between fm s0 and fm s1
NWARM_POST = 3     # PE filler between fm s1 and first mains

_CACHE = {}


def _bc_list(s):
    return BC.get(s, BC_STEADY)


def _gp_list(s):
    return GP.get(s, GP_STEADY)


def _emit_list(s):
    return EMIT.get(s, EMIT_STEADY)


def _build():
    nc = bacc.Bacc("TRN2", target_bir_lowering=False, debug=False,
                   num_devices=N_CORES)

    x_d = nc.dram_tensor("x", [C, P], BF16, kind="ExternalInput")
    w1s_d = nc.dram_tensor("w1s", [C, NUMS], BF16, kind="ExternalInput")
    w2t_d = nc.dram_tensor("w2t", [C, NUMS * OUT], BF16, kind="ExternalInput")
    b1_d = nc.dram_tensor("b1c", [NUMS, 1], F32, kind="ExternalInput")
    b2_d = nc.dram_tensor("b2c", [C, 2], F32, kind="ExternalInput")
    # c-major bf16 output: host reorders to [OUT, P] fp32
    out_d = nc.dram_tensor("out", [C, 2, P // PB, PB], BF16,
                           kind="ExternalOutput")

    relu = mybir.ActivationFunctionType.Relu
    ident = mybir.ActivationFunctionType.Identity
    mult = mybir.AluOpType.mult

    W2A_G = 6   # groups 0..5 in the first W2 load

    with tile.TileContext(nc) as tc:
        with (
            tc.tile_pool(name="const", bufs=1) as cpool,
            tc.tile_pool(name="rep1", bufs=1) as rep1p,
            tc.tile_pool(name="rep2", bufs=2) as rep2p,
            tc.tile_pool(name="rep2c", bufs=1) as rep2cp,
            tc.tile_pool(name="fmgp", bufs=2) as fmgpp,
            tc.tile_pool(name="gp512", bufs=4) as gp512p,
            tc.tile_pool(name="gp1024", bufs=4) as gp1024p,
            tc.tile_pool(name="ft4", bufs=4) as ftp4,
            tc.tile_pool(name="ftm5", bufs=1) as ftp_m5,
            tc.tile_pool(name="ft2", bufs=2) as ftp2,
            tc.tile_pool(name="osb", bufs=3) as osbp,
            tc.tile_pool(name="ps", bufs=8, space="PSUM") as ps,
            tc.tile_pool(name="dr", bufs=1, space="DRAM") as drp,
        ):
            # ---- t=0: scratch memset, act-table preload, PE warmup ----
            scratch = cpool.tile([C, PB], BF16)
            nc.vector.memset(scratch[:], 0.0)
            dummy = cpool.tile([NUMS, 1], BF16)
            nc.scalar.activation(dummy[:], scratch[0:NUMS, 0:1], relu)

            ps_w = ps.tile([C, PB], F32, tag="ps", name="ps_warm")

            def warm(n):
                for _ in range(n):
                    nc.tensor.matmul(ps_w[:], scratch[:, 0:C], scratch[:],
                                     start=True, stop=True)

            warm(NWARM_PRE)

            # ---- params (scalar queue) ----
            w1s_t = cpool.tile([C, NUMS], BF16)
            nc.scalar.dma_start(w1s_t[:], w1s_d[:])
            b1_t = cpool.tile([NUMS, 1], F32)
            nc.scalar.dma_start(b1_t[:], b1_d[:])

            # x subchunk loads: first three up front on the sync queue
            x2s = [None] * NSUB

            def load_x(s):
                px0, w = SUBS[s]
                x2 = cpool.tile([C, w], BF16, tag=f"x2_{s}", name=f"x2_{s}")
                x2s[s] = x2
                nc.sync.dma_start(x2[:], x_d[:, px0:px0 + w])

            load_x(0)
            load_x(1)
            load_x(2)

            w2a = cpool.tile([C, W2A_G * 2 * C], BF16)
            nc.scalar.dma_start(w2a[:], w2t_d[:, 0:W2A_G * 2 * C])
            w2b = cpool.tile([C, (NUMS - W2A_G) * 2 * C], BF16)
            nc.scalar.dma_start(w2b[:], w2t_d[:, W2A_G * 2 * C:])
            b2_t = cpool.tile([C, 2], F32)
            nc.scalar.dma_start(b2_t[:], b2_d[:])

            def w2(g, oc):
                i = 2 * g + oc
                if g < W2A_G:
                    return w2a[:, i * C:(i + 1) * C]
                i -= 2 * W2A_G
                return w2b[:, i * C:(i + 1) * C]

            # ---- fm compute (PE matmul + scalar relu) ----
            fm_sbs = [None] * NSUB
            fm_drs = [None] * NSUB
            fm_gps = [None] * NSUB
            GP_LO = min(GP_STEADY)      # 11

            def fm_compute(s):
                px0, w = SUBS[s]
                fm_sb = cpool.tile([NUMS, w], BF16, tag=f"fm{s}",
                                   name=f"fm{s}")
                fm_sbs[s] = fm_sb
                for h0 in range(0, w, PB):
                    ps_fm = ps.tile([NUMS, PB], F32, tag="ps",
                                    name=f"psfm{s}_{h0}")
                    nc.tensor.matmul(ps_fm[:], w1s_t[:],
                                     x2s[s][:, h0:h0 + PB],
                                     start=True, stop=True)
                    nc.scalar.activation(fm_sb[:, h0:h0 + PB], ps_fm[:],
                                         relu, bias=b1_t[:])
                # pack gp rows 11..15 to partition 0 (partition_broadcast
                # can only read partition 0); one tiny SBUF->SBUF DMA
                ngp = NUMS - GP_LO
                fm_gp = fmgpp.tile([1, ngp * w], BF16, tag=f"fmgp{w}",
                                   name=f"fmgp{s}")
                fm_gps[s] = fm_gp
                nc.scalar.dma_start(fm_gp[:], fm_sb[GP_LO:NUMS, :])
                fm_drs[s] = drp.tile([NUMS, w], BF16, tag=f"fmdr{s}",
                                     name=f"fmdr{s}")

            fm_compute(0)
            warm(NWARM_MID)
            fm_compute(1)
            warm(NWARM_POST)

            # ---- per-sub broadcast / feat / mains / store ----
            def run_sub(s):
                px0, w = SUBS[s]
                chunks = _bc_list(s)
                gps = _gp_list(s)

                # fm -> DRAM, then fused broadcast DMAs (sync queue)
                nc.sync.dma_start(fm_drs[s][:], fm_sbs[s][:])
                creps = []
                for ci, (glo, ghi) in enumerate(chunks):
                    ng = ghi - glo
                    if s < 2:
                        pool = rep1p
                    else:
                        pool = rep2cp if ci == len(chunks) - 1 else rep2p
                    rep = pool.tile([C, ng * w], BF16,
                                    tag=f"bc{min(s,2)}_{ci}",
                                    name=f"bc{s}_{ci}")
                    src = (fm_drs[s][:]
                           .flatten()
                           .unsqueeze(0)[0:1, glo * w:ghi * w]
                           .broadcast_to((C, ng * w)))
                    nc.sync.dma_start(rep[:], src)
                    creps.append(rep)

                # late x loads slot into the sync queue after this sub's
                # broadcast issues (their transfers fill DMA idle slots)
                if s == 0:
                    load_x(3)
                if s == 1:
                    load_x(4)

                # feat multiplies in readiness order
                fts = {}    # g -> (ft_ap_3d, idx)
                nemit = 0
                for kind, a in _emit_list(s):
                    if kind == 'g':
                        g = a
                        pool = gp512p if w == 512 else gp1024p
                        rep = pool.tile([C, w], BF16, tag=f"gp{w}",
                                        name=f"gp{s}_{g}")
                        if g >= GP_LO:
                            src = fm_gps[s][0:1,
                                            (g - GP_LO) * w:(g - GP_LO + 1) * w]
                        else:
                            assert g == 0, "only row 0 readable in-place"
                            src = fm_sbs[s][0:1, :]
                        nc.gpsimd.partition_broadcast(rep[:], src)
                        ft = ftp4.tile([C, 1, w], BF16, tag=f"ft1_{w}",
                                       name=f"ft{s}_g{g}")
                        xin = x2s[s][:].unsqueeze(1)
                        rin = rep[:].unsqueeze(1)
                        nc.vector.tensor_tensor(ft[:], xin, rin, op=mult)
                        fts[g] = (ft, 0)
                    else:
                        glo, ghi = chunks[a]
                        ng = ghi - glo
                        rep = creps[a]
                        if ng == 1 or (ng == 2 and w == 512):
                            pool = ftp4
                        elif w == 512:
                            pool = ftp_m5
                        else:
                            pool = ftp2
                        ft = pool.tile([C, ng, w], BF16, tag=f"ft{ng}_{w}",
                                       name=f"ft{s}_c{a}")
                        xin = (x2s[s][:].unsqueeze(1)
                               .broadcast_to((C, ng, w)))
                        rin = rep[:].rearrange("c (g p) -> c g p", g=ng)
                        nc.vector.tensor_tensor(ft[:], xin, rin, op=mult)
                        for g in range(glo, ghi):
                            fts[g] = (ft, g - glo)
                    nemit += 1

                # main matmuls
                npbs = w // PB
                pso = {}
                for pb in range(npbs):
                    for oc in range(2):
                        pso[(pb, oc)] = ps.tile([C, PB], F32, tag="ps",
                                                name=f"pso{s}_{pb}_{oc}")
                for g in range(NUMS):
                    ft, idx = fts[g]
                    for pb in range(npbs):
                        px = slice(pb * PB, (pb + 1) * PB)
                        for oc in range(2):
                            nc.tensor.matmul(
                                pso[(pb, oc)][:], w2(g, oc),
                                ft[:, idx:idx + 1, px],
                                start=(g == 0), stop=(g == NUMS - 1))

                # bias + bf16 store, one DMA per pixel block (scalar queue)
                for pb in range(npbs):
                    gpb = px0 // PB + pb
                    ot = osbp.tile([C, 2, PB], BF16, tag="ot",
                                   name=f"ot{s}_{pb}")
                    for oc in range(2):
                        nc.scalar.activation(
                            ot[:, oc:oc + 1, :], pso[(pb, oc)][:],
                            ident, bias=b2_t[:, oc:oc + 1])
                    nc.scalar.dma_start(out_d[:, :, gpb, :], ot[:])

            for s in range(NSUB):
                run_sub(s)
                if s + 2 < NSUB:
                    fm_compute(s + 2)

    nc.compile()
    return nc


def _prep_params(W1, b1, W2, b2):
    bf = ml_dtypes.bfloat16
    # w1s[c, g] = W1[g, c - 8g] for 8g <= c < 8(g+1), else 0
    w1s = np.zeros((C, NUMS), dtype=bf)
    for g in range(NUMS):
        w1s[g * HEADS:(g + 1) * HEADS, g] = W1[g].astype(bf)
    # w2t[k, (g*2+oc)*128 + m] = W2[oc*128 + m, g*128 + k]
    w2t = (
        np.asarray(W2, dtype=np.float32)
        .reshape(2, C, NUMS, C)          # [oc, m, g, k]
        .transpose(3, 2, 0, 1)           # [k, g, oc, m]
        .reshape(C, NUMS * OUT)
        .astype(bf)
    )
    b1c = np.asarray(b1, dtype=np.float32).reshape(NUMS, 1).copy()
    b2c = np.asarray(b2, dtype=np.float32).reshape(2, C).T.copy()
    return w1s, w2t, b1c, b2c


def kernel(x, W1, b1, W2, b2, _trace=False, _trace_kwargs=None):
    if "nc" not in _CACHE:
        _CACHE["nc"] = _build()
    nc = _CACHE["nc"]

    w1s, w2t, b1c, b2c = _prep_params(W1, b1, W2, b2)
    xs = np.ascontiguousarray(
        np.asarray(x, dtype=np.float32).reshape(B, C, P).astype(ml_dtypes.bfloat16))
    in_maps = [
        {"x": xs[b_], "w1s": w1s, "w2t": w2t, "b1c": b1c, "b2c": b2c}
        for b_ in range(N_CORES)
    ]
    kwargs = {}
    if _trace:
        kwargs["trace"] = True
        kwargs.update(_trace_kwargs or {})
    res = run_bass_kernel_spmd(nc, in_maps, core_ids=list(range(N_CORES)),
                               **kwargs)
    # out buffer is [C, 2, P//PB, PB] bf16 c-major; reorder to [OUT, P] fp32
    out = np.stack([
        np.asarray(res.results[b_]["out"])
        .transpose(1, 0, 2, 3).reshape(OUT, P)
        for b_ in range(N_CORES)
    ]).astype(np.float32)
    out = out.reshape(B, OUT, H, W)
    if _trace:
        _CACHE["last_result"] = res
    return out


# revision 13
# speedup vs baseline: 1.1203x; 1.1203x over previous
"""Trainium2 Bass kernel for nn_CrossChannelAttention.

Reference computation (per batch b, pixel p, with C=128 channels, NUMS=16
groups of HEADS=8 channels, OUT=256):
    fm[g,p]  = relu(sum_h W1[g,h] * x[8g+h, p] + b1[g])          # [16, P]
    feat[(g,d), p] = fm[g,p] * x[d,p]                            # [2048, P]
    out[o,p] = sum_c W2[o,c] * feat[c,p] + b2[o]                 # [256, P]

Data-parallel over batch B=8 across the 8 NeuronCores (one image per core,
params replicated).  Per core:
  - PE floor: 256 accumulating K=128 N=512 bf16 matmuls (~55us at the warm
    2.4 GHz p-state).  Dummy warmup matmuls on a memset scratch tile ramp the
    p-state (3us of continuous PE busy -> 2.4 GHz) while input DMAs run, so
    the real matmuls never execute at the cold 1.2 GHz rate.
  - fm rows are broadcast to 128 partitions in [128,1024] chunks, split
    between DRAM->SBUF broadcast DMAs (13 groups; wide shapes fan out across
    all 16 DMA engines) and gpsimd.partition_broadcast (groups 12-14, read
    from a packed partition-0 SBUF row copied by one tiny DMA per chunk,
    skipping the DRAM round-trip).
  - feat = x * fm_rep on the vector engine as pure-SBUF bf16 multiplies,
    pipelined LOOKAHEAD units ahead of the consuming matmuls.
  - head ordering: all four x chunks plus the split W2 halves are issued
    before any relu-gated fm write, so no input load ever queues behind the
    first broadcast chain (the previous version lost ~12us to this).
  - outputs are written bf16 (host casts to fp32), fused [oc0|oc1] per
    pixel block: half the store traffic and a short tail.
Accuracy: bf16 matmuls with fp32 PSUM accumulation; rel err ~4e-3.
"""

import numpy as np
import ml_dtypes

import concourse.bacc as bacc
import concourse.tile as tile
from concourse import mybir
from concourse.bass_utils import run_bass_kernel_spmd

F32 = mybir.dt.float32
BF16 = mybir.dt.bfloat16

B, C, H, W = 8, 128, 64, 64
NUMS, HEADS, OUT = 16, 8, 256
P = H * W          # 4096 pixels per image
PB = 512           # pixel block (one PSUM bank of fp32)
GRP = 1024         # broadcast chunk (2 pixel blocks)
NGRP = P // GRP    # 4 broadcast groups
N_CORES = 8
LOOKAHEAD = 8      # broadcast/feat pipeline depth (in (g,k) units)
GP_GS = (12, 13, 14)   # groups replicated via gpsimd.partition_broadcast
GP_LO, GP_HI = GP_GS[0], GP_GS[-1] + 1
W2A_G = 6          # groups 0..5 in the first W2 load

NWARM_PRE = 5      # PE p-state ramp matmuls before fm k0
NWARM_MID = 2      # PE filler between fm chunks
NWARM_POST = 9     # PE filler between last fm and first main

_CACHE = {}


def _build():
    nc = bacc.Bacc("TRN2", target_bir_lowering=False, debug=False,
                   num_devices=N_CORES)

    x_d = nc.dram_tensor("x", [C, P], BF16, kind="ExternalInput")
    w1s_d = nc.dram_tensor("w1s", [C, NUMS], BF16, kind="ExternalInput")
    w2t_d = nc.dram_tensor("w2t", [C, NUMS * OUT], BF16, kind="ExternalInput")
    b1_d = nc.dram_tensor("b1c", [NUMS, 1], F32, kind="ExternalInput")
    b2_d = nc.dram_tensor("b2c", [C, 2], F32, kind="ExternalInput")
    # c-major bf16 output: host reorders to [OUT, P] fp32
    out_d = nc.dram_tensor("out", [C, 2, P // PB, PB], BF16,
                           kind="ExternalOutput")

    relu = mybir.ActivationFunctionType.Relu
    ident = mybir.ActivationFunctionType.Identity
    mult = mybir.AluOpType.mult

    with tile.TileContext(nc) as tc:
        with (
            tc.tile_pool(name="const", bufs=1) as cpool,
            tc.tile_pool(name="fmgp", bufs=2) as fmgpp,
            tc.tile_pool(name="repp", bufs=18) as repp,
            tc.tile_pool(name="gprep", bufs=5) as gprepp,
            tc.tile_pool(name="feat", bufs=2 * LOOKAHEAD + 2) as featp,
            tc.tile_pool(name="osb", bufs=3) as osbp,
            tc.tile_pool(name="ps", bufs=8, space="PSUM") as ps,
            tc.tile_pool(name="dr", bufs=1, space="DRAM") as drp,
        ):
            # ---- t=0: memset scratch, preload act table, ramp PE ----
            scratch = cpool.tile([C, PB], BF16)
            nc.vector.memset(scratch[:], 0.0)

            ps_w = ps.tile([C, PB], F32, tag="ps", name="ps_warm")

            def warm(n):
                for _ in range(n):
                    nc.tensor.matmul(ps_w[:], scratch[:, 0:C], scratch[:],
                                     start=True, stop=True)

            warm(NWARM_PRE)

            # scalar queue: w1s/b1 first (fm needs them), then act-table
            # preload, then the split W2 + b2
            w1s_t = cpool.tile([C, NUMS], BF16)
            nc.scalar.dma_start(w1s_t[:], w1s_d[:])
            b1_t = cpool.tile([NUMS, 1], F32)
            nc.scalar.dma_start(b1_t[:], b1_d[:])
            dummy = cpool.tile([NUMS, 1], BF16)
            nc.scalar.activation(dummy[:], scratch[0:NUMS, 0:1], relu)
            w2a = cpool.tile([C, W2A_G * 2 * C], BF16)
            nc.scalar.dma_start(w2a[:], w2t_d[:, 0:W2A_G * 2 * C])
            w2b = cpool.tile([C, (NUMS - W2A_G) * 2 * C], BF16)
            nc.scalar.dma_start(w2b[:], w2t_d[:, W2A_G * 2 * C:])
            b2_t = cpool.tile([C, 2], F32)
            nc.scalar.dma_start(b2_t[:], b2_d[:])

            def w2(g, oc):
                i = 2 * g + oc
                if g < W2A_G:
                    return w2a[:, i * C:(i + 1) * C]
                i -= 2 * W2A_G
                return w2b[:, i * C:(i + 1) * C]

            # sync queue: all four x chunks up front
            x2s = []
            for k in range(NGRP):
                x2 = cpool.tile([C, GRP], BF16, tag=f"x2_{k}", name=f"x2_{k}")
                x2s.append(x2)
                nc.sync.dma_start(x2[:], x_d[:, k * GRP:(k + 1) * GRP])

            # ---- fm: matmul + relu per k chunk, PE fillers between ----
            fm_sb = cpool.tile([NUMS, P], BF16)
            fm_drs = [drp.tile([NUMS, GRP], BF16, tag=f"fmdr{k}",
                               name=f"fmdr{k}") for k in range(NGRP)]
            fm_gps = []

            for k in range(NGRP):
                for half in range(2):
                    pb = 2 * k + half
                    px = slice(pb * PB, (pb + 1) * PB)
                    hx = slice(half * PB, (half + 1) * PB)
                    ps_fm = ps.tile([NUMS, PB], F32, tag="ps",
                                    name=f"psfm{pb}")
                    nc.tensor.matmul(ps_fm[:], w1s_t[:], x2s[k][:, hx],
                                     start=True, stop=True)
                    nc.scalar.activation(fm_sb[:, px], ps_fm[:], relu,
                                         bias=b1_t[:])
                if k < NGRP - 1:
                    warm(NWARM_MID)
            warm(NWARM_POST)

            # ---- fm writes + gp packs (per k) ----
            def emit_fm_write(k):
                gx = slice(k * GRP, (k + 1) * GRP)
                nc.sync.dma_start(fm_drs[k][:], fm_sb[:, gx])
                fm_gp = fmgpp.tile([1, len(GP_GS) * GRP], BF16, tag="fmgp",
                                   name=f"fmgp{k}")
                fm_gps.append(fm_gp)
                nc.scalar.dma_start(fm_gp[:], fm_sb[GP_LO:GP_HI, gx])

            emit_fm_write(0)

            # ---- replication + feat, pipelined ahead of the mains ----
            nbc = [0]

            def emit_rep_grp(g, k):
                if g in GP_GS:
                    rep = gprepp.tile([C, GRP], BF16, tag="gprep",
                                      name=f"rep{g}_{k}")
                    src = fm_gps[k][0:1,
                                    (g - GP_LO) * GRP:(g - GP_LO + 1) * GRP]
                    nc.gpsimd.partition_broadcast(rep[:], src)
                else:
                    rep = repp.tile([C, GRP], BF16, tag="rep",
                                    name=f"rep{g}_{k}")
                    eng = nc.sync if nbc[0] % 3 != 2 else nc.scalar
                    nbc[0] += 1
                    eng.dma_start(rep[:],
                                  fm_drs[k][g:g + 1, :].broadcast_to((C, GRP)))
                return rep

            fts = {}      # (g, k) -> [C, GRP] feat tile

            def emit_ft(g, k):
                rep = emit_rep_grp(g, k)
                ft = featp.tile([C, GRP], BF16, tag="ft", name=f"ft{g}_{k}")
                nc.vector.tensor_tensor(ft[:], x2s[k][:], rep[:], op=mult)
                fts[(g, k)] = ft

            todo = [(g, k) for k in range(NGRP) for g in range(NUMS)]
            for i in range(LOOKAHEAD):
                emit_ft(*todo[i])

            pso = {}
            for i, (g, k) in enumerate(todo):
                if g == 0 and k + 1 < NGRP:
                    emit_fm_write(k + 1)
                if i + LOOKAHEAD < len(todo):
                    emit_ft(*todo[i + LOOKAHEAD])
                ft = fts.pop((g, k))
                if g == 0:
                    for pbb in (2 * k, 2 * k + 1):
                        for oc in range(2):
                            t = ps.tile([C, PB], F32, tag="ps",
                                        name=f"pso{pbb}_{oc}")
                            pso[(pbb, oc)] = t
                for half in range(2):
                    pb = 2 * k + half
                    hx = slice(half * PB, (half + 1) * PB)
                    for oc in range(2):
                        nc.tensor.matmul(pso[(pb, oc)][:], w2(g, oc),
                                         ft[:, hx], start=(g == 0),
                                         stop=(g == NUMS - 1))
                if g == NUMS - 1:
                    for pbb in (2 * k, 2 * k + 1):
                        ot = osbp.tile([C, 2, PB], BF16, tag="ot",
                                       name=f"ot{pbb}")
                        for oc in range(2):
                            nc.scalar.activation(
                                ot[:, oc:oc + 1, :],
                                pso.pop((pbb, oc))[:], ident,
                                bias=b2_t[:, oc:oc + 1])
                        nc.scalar.dma_start(out_d[:, :, pbb, :], ot[:])

    nc.compile()
    return nc


def _prep_params(W1, b1, W2, b2):
    bf = ml_dtypes.bfloat16
    # w1s[c, g] = W1[g, c - 8g] for 8g <= c < 8(g+1), else 0
    w1s = np.zeros((C, NUMS), dtype=bf)
    for g in range(NUMS):
        w1s[g * HEADS:(g + 1) * HEADS, g] = W1[g].astype(bf)
    # w2t[k, (g*2+oc)*128 + m] = W2[oc*128 + m, g*128 + k]
    w2t = (
        np.asarray(W2, dtype=np.float32)
        .reshape(2, C, NUMS, C)          # [oc, m, g, k]
        .transpose(3, 2, 0, 1)           # [k, g, oc, m]
        .reshape(C, NUMS * OUT)
        .astype(bf)
    )
    b1c = np.asarray(b1, dtype=np.float32).reshape(NUMS, 1).copy()
    b2c = np.asarray(b2, dtype=np.float32).reshape(2, C).T.copy()
    return w1s, w2t, b1c, b2c


def kernel(x, W1, b1, W2, b2, _trace=False, _trace_kwargs=None):
    if "nc" not in _CACHE:
        _CACHE["nc"] = _build()
    nc = _CACHE["nc"]

    w1s, w2t, b1c, b2c = _prep_params(W1, b1, W2, b2)
    xs = np.ascontiguousarray(
        np.asarray(x, dtype=np.float32).reshape(B, C, P).astype(ml_dtypes.bfloat16))
    in_maps = [
        {"x": xs[b_], "w1s": w1s, "w2t": w2t, "b1c": b1c, "b2c": b2c}
        for b_ in range(N_CORES)
    ]
    kwargs = {}
    if _trace:
        kwargs["trace"] = True
        kwargs.update(_trace_kwargs or {})
    res = run_bass_kernel_spmd(nc, in_maps, core_ids=list(range(N_CORES)),
                               **kwargs)
    # out buffer is [C, 2, P//PB, PB] bf16 c-major; reorder to [OUT, P] fp32
    out = np.stack([
        np.asarray(res.results[b_]["out"])
        .transpose(1, 0, 2, 3).reshape(OUT, P)
        for b_ in range(N_CORES)
    ]).astype(np.float32)
    out = out.reshape(B, OUT, H, W)
    if _trace:
        _CACHE["last_result"] = res
    return out
